# revision 33
# baseline (speedup 1.0000x reference)
"""Causal multi-head attention for TRN2, 8 NeuronCores.

Problem: x[4, 2048, 768], 12 heads of d_head=64 (W_Q/K/V [12, 768, 64],
W_O [12, 64, 768]), causal softmax attention, out [4, 2048, 768].

Sharding: 48 (batch, head) pairs -> 8 cores get (batch b = core//2,
6 heads = one half of the 12). Each core computes sum_{its heads}
z_n @ W_O[n] for its batch -> partial out^T [768, 2048]; the host adds
the two partials per batch and transposes back.

Per-core kernel (all matmuls in float32r: fp32 data, ~tf32 matmul
precision at full PE speed):
  xT [768, 2048] resident in SBUF.
  V[s, h] per head (seq-major, with a ones column at h=64 so the
  z-matmul also produces the softmax denominator).
  Per head pair p: Q^T, K^T [128, 2048] (two heads stacked on
  partitions) = W^T @ xT, bias via tensor_scalar on the PSUM->SBUF copy.
  Per head, per 512-wide q chunk: S^T[k,q] blocks = K^T.T @ Q^T
  (contraction h=64; the pair's two heads sit on partition bases 0/64
  -> disjoint PE row groups, interleaved so the hardware overlaps
  them), exp via ACT (scale=1/8 folds in 1/sqrt(d_head)). Blocks
  strictly above the causal diagonal are skipped; blocks on the
  diagonal are computed/exp'd only on their valid column range and the
  single triangular 128x128 sub-block is masked with one DVE multiply.
  z^T[h,q] (+ colsum row) = [V|1].T @ expS^T accumulated over k chunks.
  Softmax normalization: reciprocal of the colsum row, broadcast
  across partitions with a K=1 matmul against ones, then
  copy + multiply into the Z stack [128, 3, 2048].
  Output: out^T[m, s] = W_O_stack.T @ Z (contraction 384 = 3x128),
  folded bias c = b_O + sum_n b_V[n] @ W_O[n] added on the PSUM->SBUF
  copy (per-partition in the out^T layout), DMA to DRAM.
"""

import numpy as np

import concourse.mybir as mybir
import concourse.tile as tile
from concourse import bacc
from concourse.bass_utils import run_bass_kernel_spmd

B, S, DM, NH, DH = 4, 2048, 768, 12, 64
P = 128
HPC = NH // 2            # heads per core: 6
NPAIR = HPC // 2         # head pairs per core: 3
MC = DM // P             # 6 contraction chunks of 128 over d_model
QC = S // 512            # 4 q chunks of 512
KC = S // P              # 16 k chunks of 128
NCORES = 8

F32 = mybir.dt.float32
F32R = mybir.dt.float32r


def build_nc(reps: int = 1, dyn_reps: int | None = None):
    nc = bacc.Bacc(None, target_bir_lowering=False)

    xT = nc.dram_tensor("xT", [DM, S], F32R, kind="ExternalInput")
    wq = nc.dram_tensor("wq", [DM, HPC * DH], F32R, kind="ExternalInput")
    wk = nc.dram_tensor("wk", [DM, HPC * DH], F32R, kind="ExternalInput")
    wv = nc.dram_tensor("wv", [DM, HPC * DH], F32R, kind="ExternalInput")
    wo = nc.dram_tensor("wo", [HPC * DH, DM], F32R, kind="ExternalInput")
    # packed per-partition vectors: b_Q pairs (3), b_K pairs (3), c bias (6)
    bias = nc.dram_tensor("bias", [P, 2 * NPAIR + MC], F32, kind="ExternalInput")
    # msk[k, 0:128] = tril ones (k <= q); msk[k, 128] = 1.0 everywhere
    msk = nc.dram_tensor("msk", [P, P + 1], F32R, kind="ExternalInput")
    out = nc.dram_tensor("out", [DM, S], F32, kind="ExternalOutput")

    with tile.TileContext(nc) as tc:
        with (
            tc.tile_pool(name="fx", bufs=1) as fx,
            tc.tile_pool(name="qk", bufs=2) as qkp,
            tc.tile_pool(name="es", bufs=2) as esp,
            tc.tile_pool(name="sm", bufs=2) as smp,
            tc.tile_pool(name="ot", bufs=2) as otp,
            tc.tile_pool(name="psS", bufs=1, space="PSUM") as psS,
            tc.tile_pool(name="psZ", bufs=1, space="PSUM") as psZ,
            tc.tile_pool(name="psP", bufs=1, space="PSUM") as psP,
            tc.tile_pool(name="psB", bufs=1, space="PSUM") as psB,
        ):
            def emit_body():
                # ---- resident tiles + input DMA ----
                # xT lands in q-quarter-major order: the first 1.5MB
                # (all mc of q chunk 0) unblocks pair-0 Q/K projection and
                # the first attention blocks ~3x earlier than whole-row DMAs.
                xT_sb = fx.tile([P, MC, S], F32R, tag="xT")
                for q4 in range(QC):
                    for mc in range(MC):
                        nc.sync.dma_start(
                            xT_sb[:, mc, q4 * 512:(q4 + 1) * 512],
                            xT[mc * P:(mc + 1) * P, q4 * 512:(q4 + 1) * 512],
                        )
                wq_sb = fx.tile([P, MC, HPC * DH], F32R, tag="wq")
                wk_sb = fx.tile([P, MC, HPC * DH], F32R, tag="wk")
                wv_sb = fx.tile([P, MC, HPC * DH], F32R, tag="wv")
                for w_sb, w_dr in ((wq_sb, wq), (wv_sb, wv), (wk_sb, wk)):
                    nc.gpsimd.dma_start(
                        w_sb[:], w_dr.rearrange("(c p) h -> p c h", p=P)
                    )
                wo_sb = fx.tile([P, NPAIR, DM], F32R, tag="wo")
                nc.gpsimd.dma_start(wo_sb[:], wo.rearrange("(c p) m -> p c m", p=P))
                bias_sb = fx.tile([P, 2 * NPAIR + MC], F32, tag="bias")
                nc.gpsimd.dma_start(bias_sb[:], bias[:])
                bq_sb = bias_sb[:, 0:NPAIR]
                bk_sb = bias_sb[:, NPAIR:2 * NPAIR]
                cb_sb = bias_sb[:, 2 * NPAIR:]
                msk_sb = fx.tile([P, P + 1], F32R, tag="msk")
                nc.gpsimd.dma_start(msk_sb[:], msk[:])

                V_all = fx.tile([P, KC, HPC, DH + 1], F32R, tag="V")
                nc.vector.tensor_copy(
                    V_all[:, :, :, DH],
                    msk_sb[:, P:P + 1, None].to_broadcast([P, KC, HPC]),
                )
                Z = fx.tile([P, NPAIR, S], F32R, tag="Z")

                def qk_proj(p):
                    qt = qkp.tile([P, S], F32R, tag="qt", name="qt")
                    kt = qkp.tile([P, S], F32R, tag="kt", name="kt")
                    for t_sb, w_sb, b_sb in (
                        (qt, wq_sb, bq_sb), (kt, wk_sb, bk_sb)
                    ):
                        for qc in range(QC):
                            pj = psP.tile([P, 512], F32, tag="pj", name="pj")
                            for mc in range(MC):
                                nc.tensor.matmul(
                                    pj[:],
                                    w_sb[:, mc, 2 * p * DH:(2 * p + 2) * DH],
                                    xT_sb[:, mc, qc * 512:(qc + 1) * 512],
                                    start=(mc == 0),
                                    stop=(mc == MC - 1),
                                )
                            nc.vector.tensor_scalar_add(
                                t_sb[:, qc * 512:(qc + 1) * 512],
                                pj[:],
                                b_sb[:, p:p + 1],
                            )
                    return qt, kt

                # pair 0's Q/K projection first so attention can start as
                # early as possible; V projection overlaps behind it.
                qk0 = qk_proj(0)

                # ---- V projection: V[s, h] for all 6 heads ----
                for sc in range(KC):
                    pj = psP.tile([P, 512], F32, tag="pj")
                    vpj = pj[:, :HPC * DH]
                    for mc in range(MC):
                        nc.tensor.matmul(
                            vpj,
                            xT_sb[:, mc, sc * P:(sc + 1) * P],
                            wv_sb[:, mc, :],
                            start=(mc == 0),
                            stop=(mc == MC - 1),
                        )
                    nc.vector.tensor_copy(
                        V_all[:, sc, :, 0:DH],
                        vpj.rearrange("p (n h) -> p n h", n=HPC),
                    )

                def out_proj_qc(qc):
                    # out^T[m, s] for one q chunk; emitted right after the
                    # last pair finishes this qc so it overlaps the
                    # remaining attention instead of trailing at the end.
                    for mc in range(MC):
                        pj = psP.tile([P, 512], F32, tag="pj", name="pj")
                        for pp in range(NPAIR):
                            nc.tensor.matmul(
                                pj[:],
                                wo_sb[:, pp, mc * P:(mc + 1) * P],
                                Z[:, pp, qc * 512:(qc + 1) * 512],
                                start=(pp == 0),
                                stop=(pp == NPAIR - 1),
                            )
                        ot = otp.tile([P, 512], F32, tag="ot", name="ot")
                        nc.vector.tensor_scalar_add(
                            ot[:], pj[:], cb_sb[:, mc:mc + 1])
                        nc.sync.dma_start(
                            out[mc * P:(mc + 1) * P, qc * 512:(qc + 1) * 512],
                            ot[:],
                        )

                # ---- per head pair ----
                for p in range(NPAIR):
                    qt, kt = qk0 if p == 0 else qk_proj(p)

                    # attention for both heads of the pair, interleaved so
                    # the two heads' K=64 matmuls (PE row groups 0-63 /
                    # 64-127) overlap on the array.
                    for qc in range(QC):
                        nkc = 4 * qc + 4
                        zz = [psZ.tile([DH + 1, 512], F32, tag=f"z{nl}", name=f"z{nl}")
                              for nl in range(2)]
                        for g in range((nkc + 1) // 2):
                            kcs = [k for k in (2 * g, 2 * g + 1) if k < nkc]
                            # valid column start of the S^T block (causal):
                            # kc < 4qc -> 0 (full); kc = 4qc + i -> i*128
                            c0s = [max(0, (k - 4 * qc) * P) for k in kcs]
                            sgs = [psS.tile([P, 2, 512], F32, tag=f"sg{nl}", name=f"sg{nl}")
                                   for nl in range(2)]
                            ess = [esp.tile([P, 2, 512], F32R, tag=f"es{nl}", name=f"es{nl}")
                                   for nl in range(2)]
                            for j, kc in enumerate(kcs):
                                for nl in range(2):
                                    hb = nl * DH
                                    nc.tensor.matmul(
                                        sgs[nl][:, j, c0s[j]:],
                                        kt[hb:hb + DH, kc * P:(kc + 1) * P],
                                        qt[hb:hb + DH,
                                           qc * 512 + c0s[j]:(qc + 1) * 512],
                                        start=True,
                                        stop=True,
                                    )
                            for nl in range(2):
                                if len(set(c0s)) == 1:
                                    nc.scalar.activation(
                                        ess[nl][:, 0:len(kcs), c0s[0]:],
                                        sgs[nl][:, 0:len(kcs), c0s[0]:],
                                        mybir.ActivationFunctionType.Exp,
                                        scale=0.125,
                                    )
                                else:
                                    for j in range(len(kcs)):
                                        nc.scalar.activation(
                                            ess[nl][:, j, c0s[j]:],
                                            sgs[nl][:, j, c0s[j]:],
                                            mybir.ActivationFunctionType.Exp,
                                            scale=0.125,
                                        )
                            for nl in range(2):
                                for j, kc in enumerate(kcs):
                                    i = kc - 4 * qc
                                    if 0 <= i <= 3:
                                        nc.vector.tensor_tensor(
                                            ess[nl][:, j, i * P:(i + 1) * P],
                                            ess[nl][:, j, i * P:(i + 1) * P],
                                            msk_sb[:, 0:P],
                                            mybir.AluOpType.mult,
                                        )
                            for nl in range(2):
                                n = 2 * p + nl
                                for j, kc in enumerate(kcs):
                                    nc.tensor.matmul(
                                        zz[nl][:, c0s[j]:],
                                        V_all[:, kc, n, :],
                                        ess[nl][:, j, c0s[j]:],
                                        start=(kc == 0),
                                        stop=(kc == nkc - 1),
                                    )
                        # softmax normalize + write into Z stack
                        for nl in range(2):
                            hb = nl * DH
                            z = zz[nl]
                            rc = smp.tile([DH + 1, 512], F32R, tag="rc")
                            with nc.allow_low_precision(
                                reason="f32r rounding of softmax reciprocal"
                            ):
                                nc.vector.reciprocal(
                                    rc[DH:DH + 1, :], z[DH:DH + 1, :]
                                )
                            bc = psB.tile([DH, 512], F32, tag="bc")
                            nc.tensor.matmul(
                                bc[:],
                                msk_sb[DH:DH + 1, DH:2 * DH],
                                rc[DH:DH + 1, :],
                                start=True,
                                stop=True,
                            )
                            zs = Z[hb:hb + DH, p, qc * 512:(qc + 1) * 512]
                            nc.vector.tensor_copy(zs, z[0:DH, :])
                            nc.vector.tensor_tensor(
                                zs, zs, bc[:], mybir.AluOpType.mult
                            )
                        if p == NPAIR - 1:
                            out_proj_qc(qc)


            if dyn_reps is not None:
                with tc.For_i(0, dyn_reps, 1):
                    emit_body()
            else:
                for _rep in range(reps):
                    emit_body()
    nc.finalize()
    return nc


def make_in_maps(inputs):
    x = np.asarray(inputs["normalized_resid_pre"], dtype=np.float32)
    W_Q = np.asarray(inputs["W_Q"], dtype=np.float32)
    W_K = np.asarray(inputs["W_K"], dtype=np.float32)
    W_V = np.asarray(inputs["W_V"], dtype=np.float32)
    W_O = np.asarray(inputs["W_O"], dtype=np.float32)
    b_Q = np.asarray(inputs["b_Q"], dtype=np.float32)
    b_K = np.asarray(inputs["b_K"], dtype=np.float32)
    b_V = np.asarray(inputs["b_V"], dtype=np.float32)
    b_O = np.asarray(inputs["b_O"], dtype=np.float32)

    k = np.arange(P)[:, None]
    q = np.arange(P)[None, :]
    msk = np.concatenate(
        [(q >= k).astype(np.float32), np.ones((P, 1), np.float32)], axis=1
    )  # [128, 129]

    in_maps = []
    for c in range(NCORES):
        b = c // 2
        hg = (c % 2) * HPC
        hs = slice(hg, hg + HPC)
        xT_b = np.ascontiguousarray(x[b].T)  # [768, 2048]
        wq_c = np.ascontiguousarray(
            W_Q[hs].transpose(1, 0, 2).reshape(DM, HPC * DH))
        wk_c = np.ascontiguousarray(
            W_K[hs].transpose(1, 0, 2).reshape(DM, HPC * DH))
        wv_c = np.ascontiguousarray(
            W_V[hs].transpose(1, 0, 2).reshape(DM, HPC * DH))
        wo_c = np.ascontiguousarray(W_O[hs].reshape(HPC * DH, DM))
        bq_c = b_Q[hs].reshape(NPAIR, P).T
        bk_c = b_K[hs].reshape(NPAIR, P).T
        cvec = np.einsum("nh,nhm->m", b_V[hs], W_O[hs]).astype(np.float32)
        if hg == 0:
            cvec = cvec + b_O
        cb_c = cvec.reshape(MC, P).T
        bias_c = np.ascontiguousarray(
            np.concatenate([bq_c, bk_c, cb_c], axis=1))
        in_maps.append({
            "xT": xT_b, "wq": wq_c, "wk": wk_c, "wv": wv_c, "wo": wo_c,
            "bias": bias_c, "msk": msk,
        })
    return in_maps


def assemble(results):
    out = np.empty((B, S, DM), dtype=np.float32)
    for b in range(B):
        acc = results[2 * b]["out"] + results[2 * b + 1]["out"]  # [768, 2048]
        out[b] = acc.T
    return out


def kernel(**inputs) -> np.ndarray:
    nc = build_nc(reps=1)
    in_maps = make_in_maps(inputs)
    res = run_bass_kernel_spmd(nc, in_maps, list(range(NCORES)))
    return assemble(res.results)


# revision 34
# speedup vs baseline: 1.1299x; 1.1299x over previous
"""Causal multi-head attention for TRN2, 8 NeuronCores.

Problem: x[4, 2048, 768], 12 heads of d_head=64 (W_Q/K/V [12, 768, 64],
W_O [12, 64, 768]), causal softmax attention, out [4, 2048, 768].

Sharding: 48 (batch, head) pairs -> 8 cores get (batch b = core//2,
6 heads = one half of the 12). Each core computes sum_{its heads}
z_n @ W_O[n] for its batch -> partial out^T [768, 2048]; the host adds
the two partials per batch and transposes back.

Per-core kernel (all matmuls in float32r: fp32 data, ~tf32 matmul
precision at full PE speed):
  xT [768, 2048] resident in SBUF.
  V[s, h] per head (seq-major, with a ones column at h=64 so the
  z-matmul also produces the softmax denominator).
  Per head pair p: Q^T, K^T [128, 2048] (two heads stacked on
  partitions) = W^T @ xT, bias via tensor_scalar on the PSUM->SBUF copy.
  Per head, per 512-wide q chunk: S^T[k,q] blocks = K^T.T @ Q^T
  (contraction h=64; the pair's two heads sit on partition bases 0/64
  -> disjoint PE row groups, interleaved so the hardware overlaps
  them), exp via ACT (scale=1/8 folds in 1/sqrt(d_head)). Blocks
  strictly above the causal diagonal are skipped; blocks on the
  diagonal are computed/exp'd only on their valid column range and the
  single triangular 128x128 sub-block is masked with one DVE multiply.
  z^T[h,q] (+ colsum row) = [V|1].T @ expS^T accumulated over k chunks.
  Softmax normalization: reciprocal of the colsum row, broadcast
  across partitions with a K=1 matmul against ones, then
  copy + multiply into the Z stack [128, 3, 2048].
  Output: out^T[m, s] = W_O_stack.T @ Z (contraction 384 = 3x128),
  folded bias c = b_O + sum_n b_V[n] @ W_O[n] added on the PSUM->SBUF
  copy (per-partition in the out^T layout), DMA to DRAM.
"""

import numpy as np

import concourse.mybir as mybir
import concourse.tile as tile
from concourse import bacc
from concourse.bass_utils import run_bass_kernel_spmd

B, S, DM, NH, DH = 4, 2048, 768, 12, 64
P = 128
HPC = NH // 2            # heads per core: 6
NPAIR = HPC // 2         # head pairs per core: 3
MC = DM // P             # 6 contraction chunks of 128 over d_model
QC = S // 512            # 4 q chunks of 512
KC = S // P              # 16 k chunks of 128
NCORES = 8

F32 = mybir.dt.float32
F32R = mybir.dt.float32r


def build_nc(reps: int = 1, dyn_reps: int | None = None):
    nc = bacc.Bacc(None, target_bir_lowering=False)

    xT = nc.dram_tensor("xT", [DM, S], F32R, kind="ExternalInput")
    wq = nc.dram_tensor("wq", [DM, HPC * DH], F32R, kind="ExternalInput")
    wk = nc.dram_tensor("wk", [DM, HPC * DH], F32R, kind="ExternalInput")
    wv = nc.dram_tensor("wv", [DM, HPC * DH], F32R, kind="ExternalInput")
    wo = nc.dram_tensor("wo", [HPC * DH, DM], F32R, kind="ExternalInput")
    # packed per-partition vectors: b_Q pairs (3), b_K pairs (3), c bias (6)
    bias = nc.dram_tensor("bias", [P, 2 * NPAIR + MC], F32, kind="ExternalInput")
    # msk[k, 0:128] = tril ones (k <= q); msk[k, 128] = 1.0 everywhere
    msk = nc.dram_tensor("msk", [P, P + 1], F32R, kind="ExternalInput")
    out = nc.dram_tensor("out", [DM, S], F32, kind="ExternalOutput")

    with tile.TileContext(nc) as tc:
        with (
            tc.tile_pool(name="fx", bufs=1) as fx,
            tc.tile_pool(name="qk", bufs=2) as qkp,
            tc.tile_pool(name="es", bufs=2) as esp,
            tc.tile_pool(name="sm", bufs=2) as smp,
            tc.tile_pool(name="ot", bufs=2) as otp,
            tc.tile_pool(name="psS", bufs=1, space="PSUM") as psS,
            tc.tile_pool(name="psZ", bufs=1, space="PSUM") as psZ,
            tc.tile_pool(name="psP", bufs=1, space="PSUM") as psP,
            tc.tile_pool(name="psB", bufs=1, space="PSUM") as psB,
        ):
            def emit_body():
                # ---- resident tiles + input DMA ----
                xT_sb = fx.tile([P, MC, S], F32R, tag="xT")
                for mc in range(MC):
                    nc.sync.dma_start(
                        xT_sb[:, mc, :], xT[mc * P:(mc + 1) * P, :]
                    )
                wq_sb = fx.tile([P, MC, HPC * DH], F32R, tag="wq")
                wk_sb = fx.tile([P, MC, HPC * DH], F32R, tag="wk")
                wv_sb = fx.tile([P, MC, HPC * DH], F32R, tag="wv")
                for w_sb, w_dr in ((wq_sb, wq), (wv_sb, wv), (wk_sb, wk)):
                    nc.gpsimd.dma_start(
                        w_sb[:], w_dr.rearrange("(c p) h -> p c h", p=P)
                    )
                wo_sb = fx.tile([P, NPAIR, DM], F32R, tag="wo")
                nc.gpsimd.dma_start(wo_sb[:], wo.rearrange("(c p) m -> p c m", p=P))
                bias_sb = fx.tile([P, 2 * NPAIR + MC], F32, tag="bias")
                nc.gpsimd.dma_start(bias_sb[:], bias[:])
                bq_sb = bias_sb[:, 0:NPAIR]
                bk_sb = bias_sb[:, NPAIR:2 * NPAIR]
                cb_sb = bias_sb[:, 2 * NPAIR:]
                msk_sb = fx.tile([P, P + 1], F32R, tag="msk")
                nc.gpsimd.dma_start(msk_sb[:], msk[:])

                V_all = fx.tile([P, KC, HPC, DH + 1], F32R, tag="V")
                nc.vector.tensor_copy(
                    V_all[:, :, :, DH],
                    msk_sb[:, P:P + 1, None].to_broadcast([P, KC, HPC]),
                )
                Z = fx.tile([P, NPAIR, S], F32R, tag="Z")

                def qk_proj(p):
                    qt = qkp.tile([P, S], F32R, tag="qt", name="qt")
                    kt = qkp.tile([P, S], F32R, tag="kt", name="kt")
                    for t_sb, w_sb, b_sb in (
                        (qt, wq_sb, bq_sb), (kt, wk_sb, bk_sb)
                    ):
                        for qc in range(QC):
                            pj = psP.tile([P, 512], F32, tag="pj", name="pj")
                            for mc in range(MC):
                                nc.tensor.matmul(
                                    pj[:],
                                    w_sb[:, mc, 2 * p * DH:(2 * p + 2) * DH],
                                    xT_sb[:, mc, qc * 512:(qc + 1) * 512],
                                    start=(mc == 0),
                                    stop=(mc == MC - 1),
                                )
                            nc.vector.tensor_scalar_add(
                                t_sb[:, qc * 512:(qc + 1) * 512],
                                pj[:],
                                b_sb[:, p:p + 1],
                            )
                    return qt, kt

                # pair 0's Q/K projection first so attention can start as
                # early as possible; V projection overlaps behind it.
                qk0 = qk_proj(0)

                # ---- V projection: V[s, h] for all 6 heads ----
                for sc in range(KC):
                    pj = psP.tile([P, 512], F32, tag="pj")
                    vpj = pj[:, :HPC * DH]
                    for mc in range(MC):
                        nc.tensor.matmul(
                            vpj,
                            xT_sb[:, mc, sc * P:(sc + 1) * P],
                            wv_sb[:, mc, :],
                            start=(mc == 0),
                            stop=(mc == MC - 1),
                        )
                    nc.vector.tensor_copy(
                        V_all[:, sc, :, 0:DH],
                        vpj.rearrange("p (n h) -> p n h", n=HPC),
                    )

                def out_proj_qc(qc):
                    # out^T[m, s] for one q chunk; emitted right after the
                    # last pair finishes this qc so it overlaps the
                    # remaining attention instead of trailing at the end.
                    for mc in range(MC):
                        pj = psP.tile([P, 512], F32, tag="pj", name="pj")
                        for pp in range(NPAIR):
                            nc.tensor.matmul(
                                pj[:],
                                wo_sb[:, pp, mc * P:(mc + 1) * P],
                                Z[:, pp, qc * 512:(qc + 1) * 512],
                                start=(pp == 0),
                                stop=(pp == NPAIR - 1),
                            )
                        ot = otp.tile([P, 512], F32, tag="ot", name="ot")
                        nc.vector.tensor_scalar_add(
                            ot[:], pj[:], cb_sb[:, mc:mc + 1])
                        nc.sync.dma_start(
                            out[mc * P:(mc + 1) * P, qc * 512:(qc + 1) * 512],
                            ot[:],
                        )

                # ---- per head pair ----
                for p in range(NPAIR):
                    qt, kt = qk0 if p == 0 else qk_proj(p)

                    # attention for both heads of the pair, interleaved so
                    # the two heads' K=64 matmuls (PE row groups 0-63 /
                    # 64-127) overlap on the array.
                    for qc in range(QC):
                        nkc = 4 * qc + 4
                        zz = [psZ.tile([DH + 1, 512], F32, tag=f"z{nl}", name=f"z{nl}")
                              for nl in range(2)]
                        for g in range((nkc + 1) // 2):
                            kcs = [k for k in (2 * g, 2 * g + 1) if k < nkc]
                            # valid column start of the S^T block (causal):
                            # kc < 4qc -> 0 (full); kc = 4qc + i -> i*128
                            c0s = [max(0, (k - 4 * qc) * P) for k in kcs]
                            sgs = [psS.tile([P, 2, 512], F32, tag=f"sg{nl}", name=f"sg{nl}")
                                   for nl in range(2)]
                            ess = [esp.tile([P, 2, 512], F32R, tag=f"es{nl}", name=f"es{nl}")
                                   for nl in range(2)]
                            for j, kc in enumerate(kcs):
                                for nl in range(2):
                                    hb = nl * DH
                                    nc.tensor.matmul(
                                        sgs[nl][:, j, c0s[j]:],
                                        kt[hb:hb + DH, kc * P:(kc + 1) * P],
                                        qt[hb:hb + DH,
                                           qc * 512 + c0s[j]:(qc + 1) * 512],
                                        start=True,
                                        stop=True,
                                    )
                            for nl in range(2):
                                if len(set(c0s)) == 1:
                                    nc.scalar.activation(
                                        ess[nl][:, 0:len(kcs), c0s[0]:],
                                        sgs[nl][:, 0:len(kcs), c0s[0]:],
                                        mybir.ActivationFunctionType.Exp,
                                        scale=0.125,
                                    )
                                else:
                                    for j in range(len(kcs)):
                                        nc.scalar.activation(
                                            ess[nl][:, j, c0s[j]:],
                                            sgs[nl][:, j, c0s[j]:],
                                            mybir.ActivationFunctionType.Exp,
                                            scale=0.125,
                                        )
                            for nl in range(2):
                                for j, kc in enumerate(kcs):
                                    i = kc - 4 * qc
                                    if 0 <= i <= 3:
                                        nc.vector.tensor_tensor(
                                            ess[nl][:, j, i * P:(i + 1) * P],
                                            ess[nl][:, j, i * P:(i + 1) * P],
                                            msk_sb[:, 0:P],
                                            mybir.AluOpType.mult,
                                        )
                            for nl in range(2):
                                n = 2 * p + nl
                                for j, kc in enumerate(kcs):
                                    nc.tensor.matmul(
                                        zz[nl][:, c0s[j]:],
                                        V_all[:, kc, n, :],
                                        ess[nl][:, j, c0s[j]:],
                                        start=(kc == 0),
                                        stop=(kc == nkc - 1),
                                    )
                        # softmax normalize + write into Z stack
                        for nl in range(2):
                            hb = nl * DH
                            z = zz[nl]
                            rc = smp.tile([DH + 1, 512], F32R, tag="rc")
                            with nc.allow_low_precision(
                                reason="f32r rounding of softmax reciprocal"
                            ):
                                nc.vector.reciprocal(
                                    rc[DH:DH + 1, :], z[DH:DH + 1, :]
                                )
                            bc = psB.tile([DH, 512], F32, tag="bc")
                            nc.tensor.matmul(
                                bc[:],
                                msk_sb[DH:DH + 1, DH:2 * DH],
                                rc[DH:DH + 1, :],
                                start=True,
                                stop=True,
                            )
                            zs = Z[hb:hb + DH, p, qc * 512:(qc + 1) * 512]
                            nc.vector.tensor_copy(zs, z[0:DH, :])
                            nc.vector.tensor_tensor(
                                zs, zs, bc[:], mybir.AluOpType.mult
                            )
                        if p == NPAIR - 1:
                            out_proj_qc(qc)


            if dyn_reps is not None:
                with tc.For_i(0, dyn_reps, 1):
                    emit_body()
            else:
                for _rep in range(reps):
                    emit_body()
    nc.finalize()
    return nc


def make_in_maps(inputs):
    x = np.asarray(inputs["normalized_resid_pre"], dtype=np.float32)
    W_Q = np.asarray(inputs["W_Q"], dtype=np.float32)
    W_K = np.asarray(inputs["W_K"], dtype=np.float32)
    W_V = np.asarray(inputs["W_V"], dtype=np.float32)
    W_O = np.asarray(inputs["W_O"], dtype=np.float32)
    b_Q = np.asarray(inputs["b_Q"], dtype=np.float32)
    b_K = np.asarray(inputs["b_K"], dtype=np.float32)
    b_V = np.asarray(inputs["b_V"], dtype=np.float32)
    b_O = np.asarray(inputs["b_O"], dtype=np.float32)

    k = np.arange(P)[:, None]
    q = np.arange(P)[None, :]
    msk = np.concatenate(
        [(q >= k).astype(np.float32), np.ones((P, 1), np.float32)], axis=1
    )  # [128, 129]

    in_maps = []
    for c in range(NCORES):
        b = c // 2
        hg = (c % 2) * HPC
        hs = slice(hg, hg + HPC)
        xT_b = np.ascontiguousarray(x[b].T)  # [768, 2048]
        wq_c = np.ascontiguousarray(
            W_Q[hs].transpose(1, 0, 2).reshape(DM, HPC * DH))
        wk_c = np.ascontiguousarray(
            W_K[hs].transpose(1, 0, 2).reshape(DM, HPC * DH))
        wv_c = np.ascontiguousarray(
            W_V[hs].transpose(1, 0, 2).reshape(DM, HPC * DH))
        wo_c = np.ascontiguousarray(W_O[hs].reshape(HPC * DH, DM))
        bq_c = b_Q[hs].reshape(NPAIR, P).T
        bk_c = b_K[hs].reshape(NPAIR, P).T
        cvec = np.einsum("nh,nhm->m", b_V[hs], W_O[hs]).astype(np.float32)
        if hg == 0:
            cvec = cvec + b_O
        cb_c = cvec.reshape(MC, P).T
        bias_c = np.ascontiguousarray(
            np.concatenate([bq_c, bk_c, cb_c], axis=1))
        in_maps.append({
            "xT": xT_b, "wq": wq_c, "wk": wk_c, "wv": wv_c, "wo": wo_c,
            "bias": bias_c, "msk": msk,
        })
    return in_maps


def assemble(results):
    out = np.empty((B, S, DM), dtype=np.float32)
    for b in range(B):
        acc = results[2 * b]["out"] + results[2 * b + 1]["out"]  # [768, 2048]
        out[b] = acc.T
    return out


def kernel(**inputs) -> np.ndarray:
    nc = build_nc(reps=1)
    in_maps = make_in_maps(inputs)
    res = run_bass_kernel_spmd(nc, in_maps, list(range(NCORES)))
    return assemble(res.results)
